# revision 4
# baseline (speedup 1.0000x reference)
"""Graphene tight-binding band energies on 8 Trainium2 NeuronCores.

Math (derived from the 2x2 Hermitian closed form in the reference):
    With a = sin^2((A_LAT/4)kx), b = sin^2((3 A_CC/4)ky):
      cos(v) = 1-2a, cos(3u) = 1-2b   (half-angle; keeps ACT Sin args
                                       within +-1.82 where it is ~1 ULP)
      gp  = cos(v)*(cos(v)+cos(3u))   ; computed as gp2 = -gp/2 = (a+b-1)(1-2a)
      e   = -2*t'*(2*gp - 1)
      r   = sqrt((m^2+t^2) + 4*t^2*gp)    # m^2+|f|^2 = m^2+t^2+4t^2*gp
      out = [(e - r)*J, (e + r)*J]

Data parallel: k[4194304,2] split into 8 contiguous row slices, one per core.
Two-phase structure so ACT switches tables (trig -> sqrt) exactly once.
"""
import os
import sys

sys.path.insert(0, "/opt/trn_rl_repo")

import numpy as np

# ---- constants (hardcoded; must match reference.py) ----
EV_TO_J = 1.602176634e-19
A_LAT = 2.46e-10
T_EV, TP_EV, M_EV = 2.8, 0.1, 0.05
A_CC = A_LAT / np.sqrt(3.0)

N_K = 4_194_304
N_CORES = 8
N_CORE = N_K // N_CORES          # 524288 points per core
P = 128                          # SBUF partitions
F = 2048                         # points per partition per tile
T = N_CORE // (P * F)            # tiles per core (= 2)

SV2 = float(np.float32(A_LAT / 4.0))      # sv2 = Sin(SV2*kx)
SU2 = float(np.float32(0.75 * A_CC))      # s3u2 = Sin(SU2*ky)
SQ_SCALE = float(np.float32(-8.0 * T_EV * T_EV))   # r = Sqrt(SQ_SCALE*gp2 + SQ_BIAS)
SQ_BIAS = float(np.float32(M_EV * M_EV + T_EV * T_EV))
H_A = float(np.float32(8.0 * TP_EV * EV_TO_J))     # h = H_A*gp2 + H_B
H_B = float(np.float32(2.0 * TP_EV * EV_TO_J))
J = float(np.float32(EV_TO_J))

_CACHE = {}
LAST_RESULTS = None


def _build():
    import concourse.bacc as bacc
    import concourse.tile as tile
    from concourse import mybir

    AF = mybir.ActivationFunctionType
    ALU = mybir.AluOpType
    f32 = mybir.dt.float32

    nc = bacc.Bacc(None, target_bir_lowering=False)

    def register_const(value):
        t = nc.alloc_sbuf_tensor(f"const-f32-{value}", [128, 1], f32)
        nc.gpsimd.memset(t.ap(), value)
        nc.const_aps.aps[(f32, value)] = t.ap()

    register_const(SQ_BIAS)
    nc.all_engine_barrier()

    k = nc.dram_tensor("k", [N_CORE, 2], f32, kind="ExternalInput")
    out = nc.dram_tensor("out", [N_CORE, 2], f32, kind="ExternalOutput")
    kv = k.rearrange("(t p f) two -> t p (f two)", p=P, f=F)
    ov = out.rearrange("(t p f) two -> t p (f two)", p=P, f=F)

    with tile.TileContext(nc) as tc:
        with (
            tc.tile_pool(name="kin", bufs=2) as kin_pool,
            tc.tile_pool(name="trig", bufs=2) as trig_pool,
            tc.tile_pool(name="gp", bufs=T) as gp_pool,
            tc.tile_pool(name="phb", bufs=2) as phb_pool,
            tc.tile_pool(name="outp", bufs=2) as out_pool,
        ):
            gp2s = []
            # ---- phase A: trig table (Sin + Square) ----
            for t in range(T):
                kt = kin_pool.tile([P, 2 * F], f32, tag="kin")
                nc.sync.dma_start(kt[:], kv[t])
                kk = kt.rearrange("p (f two) -> p f two", two=2)
                sv2 = trig_pool.tile([P, F], f32, tag="sv2")
                s3u2 = trig_pool.tile([P, F], f32, tag="s3u2")
                nc.scalar.activation(sv2[:], kk[:, :, 0], AF.Sin, scale=SV2)
                nc.scalar.activation(s3u2[:], kk[:, :, 1], AF.Sin, scale=SU2)
                a, b = sv2, s3u2                                # in-place squares
                nc.scalar.activation(a[:], sv2[:], AF.Square)
                nc.scalar.activation(b[:], s3u2[:], AF.Square)
                w = b                                           # w = a+b (in-place)
                nc.vector.tensor_add(w[:], a[:], b[:])
                p = a                                           # p = 1-2a (in-place)
                nc.vector.tensor_scalar(p[:], a[:], -2.0, 1.0,
                                        ALU.mult, ALU.add)
                gp2 = gp_pool.tile([P, F], f32)
                nc.vector.scalar_tensor_tensor(gp2[:], w[:], 1.0, p[:],
                                               ALU.subtract, ALU.mult)
                gp2s.append(gp2)                                # (w-1)*p = -gp/2
            # ---- phase B: sqrt table ----
            for t in range(T):
                gp2 = gp2s[t]
                r = phb_pool.tile([P, F], f32, tag="r")
                nc.scalar.activation(r[:], gp2[:], AF.Sqrt,
                                     bias=SQ_BIAS, scale=SQ_SCALE)
                h = phb_pool.tile([P, F], f32, tag="h")
                nc.vector.tensor_scalar(h[:], gp2[:], H_A, H_B,
                                        ALU.mult, ALU.add)
                ot = out_pool.tile([P, 2 * F], f32, tag="ot")
                oo = ot.rearrange("p (f two) -> p f two", two=2)
                nc.vector.scalar_tensor_tensor(oo[:, :, 0], r[:], -J, h[:],
                                               ALU.mult, ALU.add)
                nc.vector.scalar_tensor_tensor(oo[:, :, 1], r[:], J, h[:],
                                               ALU.mult, ALU.add)
                nc.sync.dma_start(ov[t], ot[:])
    nc.compile()
    return nc


def kernel(k: np.ndarray) -> np.ndarray:
    global LAST_RESULTS
    from concourse.bass_utils import run_bass_kernel_spmd

    if "nc" not in _CACHE:
        _CACHE["nc"] = _build()
    nc = _CACHE["nc"]

    k = np.ascontiguousarray(np.asarray(k, dtype=np.float32))
    in_maps = [{"k": k[i * N_CORE:(i + 1) * N_CORE]} for i in range(N_CORES)]
    res = run_bass_kernel_spmd(nc, in_maps, list(range(N_CORES)),
                               trace=bool(os.environ.get("GRAPHENE_TRACE")))
    LAST_RESULTS = res
    return np.concatenate([res.results[i]["out"] for i in range(N_CORES)], axis=0)


# revision 5
# speedup vs baseline: 1.0782x; 1.0782x over previous
"""Graphene tight-binding band energies on 8 Trainium2 NeuronCores.

Math (derived from the 2x2 Hermitian closed form in the reference):
    With a = sin^2((A_LAT/4)kx), b = sin^2((3 A_CC/4)ky):
      cos(v) = 1-2a, cos(3u) = 1-2b   (half-angle; keeps ACT Sin args
                                       within +-1.82 where it is ~1 ULP)
      gp  = cos(v)*(cos(v)+cos(3u))   ; computed as gp2 = -gp/2 = (a+b-1)(1-2a)
      e   = -2*t'*(2*gp - 1)
      r   = sqrt((m^2+t^2) + 4*t^2*gp)    # m^2+|f|^2 = m^2+t^2+4t^2*gp
      out = [(e - r)*J, (e + r)*J]

Data parallel: k[4194304,2] split into 8 contiguous row slices, one per core.
Two-phase structure so ACT switches tables (trig -> sqrt) exactly once.
"""
import os
import sys

sys.path.insert(0, "/opt/trn_rl_repo")

import numpy as np

# ---- constants (hardcoded; must match reference.py) ----
EV_TO_J = 1.602176634e-19
A_LAT = 2.46e-10
T_EV, TP_EV, M_EV = 2.8, 0.1, 0.05
A_CC = A_LAT / np.sqrt(3.0)

N_K = 4_194_304
N_CORES = 8
N_CORE = N_K // N_CORES          # 524288 points per core
P = 128                          # SBUF partitions
F = 2048                         # points per partition per tile
T = N_CORE // (P * F)            # tiles per core (= 2)

SV2 = float(np.float32(A_LAT / 4.0))      # sv2 = Sin(SV2*kx)
SU2 = float(np.float32(0.75 * A_CC))      # s3u2 = Sin(SU2*ky)
SQ_SCALE = float(np.float32(-8.0 * T_EV * T_EV))   # r = Sqrt(SQ_SCALE*gp2 + SQ_BIAS)
SQ_BIAS = float(np.float32(M_EV * M_EV + T_EV * T_EV))
H_A = float(np.float32(8.0 * TP_EV * EV_TO_J))     # h = H_A*gp2 + H_B
H_B = float(np.float32(2.0 * TP_EV * EV_TO_J))
J = float(np.float32(EV_TO_J))

_CACHE = {}
LAST_RESULTS = None


def _build():
    import concourse.bacc as bacc
    import concourse.tile as tile
    from concourse import mybir

    AF = mybir.ActivationFunctionType
    ALU = mybir.AluOpType
    f32 = mybir.dt.float32

    nc = bacc.Bacc(None, target_bir_lowering=False)

    def register_const(value):
        t = nc.alloc_sbuf_tensor(f"const-f32-{value}", [128, 1], f32)
        nc.gpsimd.memset(t.ap(), value)
        nc.const_aps.aps[(f32, value)] = t.ap()

    register_const(SQ_BIAS)
    nc.all_engine_barrier()

    k = nc.dram_tensor("k", [N_CORE, 2], f32, kind="ExternalInput")
    out = nc.dram_tensor("out", [N_CORE, 2], f32, kind="ExternalOutput")
    kv = k.rearrange("(t p f) two -> t p (f two)", p=P, f=F)
    ov = out.rearrange("(t p f) two -> t p (f two)", p=P, f=F)

    with tile.TileContext(nc) as tc:
        with (
            tc.tile_pool(name="kin", bufs=2) as kin_pool,
            tc.tile_pool(name="trig", bufs=2) as trig_pool,
            tc.tile_pool(name="gp", bufs=T) as gp_pool,
            tc.tile_pool(name="phb", bufs=2) as phb_pool,
            tc.tile_pool(name="outp", bufs=2) as out_pool,
        ):
            FB = F // 2                     # phase-B sub-tile width
            gp2s = []
            # ---- phase A: trig table (Sin on ACT; squares split ACT/DVE) ----
            for t in range(T):
                kt = kin_pool.tile([P, 2 * F], f32, tag="kin")
                nc.sync.dma_start(kt[:], kv[t])
                kk = kt.rearrange("p (f two) -> p f two", two=2)
                sv2 = trig_pool.tile([P, F], f32, tag="sv2")
                s3u2 = trig_pool.tile([P, F], f32, tag="s3u2")
                nc.scalar.activation(sv2[:], kk[:, :, 0], AF.Sin, scale=SV2)
                nc.scalar.activation(s3u2[:], kk[:, :, 1], AF.Sin, scale=SU2)
                a = trig_pool.tile([P, F], f32, tag="a")
                nc.scalar.activation(a[:], sv2[:], AF.Square)   # a = sv2^2 (ACT)
                b = s3u2                                        # b = s3u2^2 (DVE)
                nc.vector.tensor_mul(b[:], s3u2[:], s3u2[:])
                w = b                                           # w = a+b (in-place)
                nc.vector.tensor_add(w[:], a[:], b[:])
                p = a                                           # p = 1-2a (in-place)
                nc.vector.tensor_scalar(p[:], a[:], -2.0, 1.0,
                                        ALU.mult, ALU.add)
                gp2 = gp_pool.tile([P, F], f32)
                nc.vector.scalar_tensor_tensor(gp2[:], w[:], 1.0, p[:],
                                               ALU.subtract, ALU.mult)
                gp2s.append(gp2)                                # (w-1)*p = -gp/2
            # ---- phase B: sqrt table; finer sub-tiles to stream out early ----
            for t in range(T):
                for s in range(F // FB):
                    gp2 = gp2s[t][:, s * FB:(s + 1) * FB]
                    r = phb_pool.tile([P, FB], f32, tag="r")
                    nc.scalar.activation(r[:], gp2, AF.Sqrt,
                                         bias=SQ_BIAS, scale=SQ_SCALE)
                    h = phb_pool.tile([P, FB], f32, tag="h")
                    nc.scalar.activation(h[:], gp2, AF.Copy,
                                         bias=H_B, scale=H_A)
                    ot = out_pool.tile([P, 2 * FB], f32, tag="ot")
                    oo = ot.rearrange("p (f two) -> p f two", two=2)
                    nc.vector.scalar_tensor_tensor(oo[:, :, 0], r[:], -J, h[:],
                                                   ALU.mult, ALU.add)
                    nc.vector.scalar_tensor_tensor(oo[:, :, 1], r[:], J, h[:],
                                                   ALU.mult, ALU.add)
                    nc.sync.dma_start(ov[t][:, 2 * s * FB:2 * (s + 1) * FB], ot[:])
    nc.compile()
    return nc


def kernel(k: np.ndarray) -> np.ndarray:
    global LAST_RESULTS
    from concourse.bass_utils import run_bass_kernel_spmd

    if "nc" not in _CACHE:
        _CACHE["nc"] = _build()
    nc = _CACHE["nc"]

    k = np.ascontiguousarray(np.asarray(k, dtype=np.float32))
    in_maps = [{"k": k[i * N_CORE:(i + 1) * N_CORE]} for i in range(N_CORES)]
    res = run_bass_kernel_spmd(nc, in_maps, list(range(N_CORES)),
                               trace=bool(os.environ.get("GRAPHENE_TRACE")))
    LAST_RESULTS = res
    return np.concatenate([res.results[i]["out"] for i in range(N_CORES)], axis=0)


# revision 9
# speedup vs baseline: 1.1673x; 1.0827x over previous
"""Graphene tight-binding band energies on 8 Trainium2 NeuronCores.

Math (derived from the 2x2 Hermitian closed form in the reference):
    With a = sin^2((A_LAT/4)kx), b = sin^2((3 A_CC/4)ky):
      cos(v) = 1-2a, cos(3u) = 1-2b   (half-angle; keeps ACT Sin args
                                       within +-1.82 where it is ~1 ULP)
      gp  = cos(v)*(cos(v)+cos(3u))   ; computed as gp2 = -gp/2 = (a+b-1)(1-2a)
      e   = -2*t'*(2*gp - 1)
      r   = sqrt((m^2+t^2) + 4*t^2*gp)    # m^2+|f|^2 = m^2+t^2+4t^2*gp
      out = [(e - r)*J, (e + r)*J]

Data parallel: k[4194304,2] split into 8 contiguous row slices, one per core.
Two-phase structure so ACT switches tables (trig -> sqrt) exactly once.
"""
import os
import sys

sys.path.insert(0, "/opt/trn_rl_repo")

import numpy as np

# ---- constants (hardcoded; must match reference.py) ----
EV_TO_J = 1.602176634e-19
A_LAT = 2.46e-10
T_EV, TP_EV, M_EV = 2.8, 0.1, 0.05
A_CC = A_LAT / np.sqrt(3.0)

N_K = 4_194_304
N_CORES = 8
N_CORE = N_K // N_CORES          # 524288 points per core
P = 128                          # SBUF partitions
F = 1024                         # points per partition per tile
T = N_CORE // (P * F)            # tiles per core (= 2)

SV2 = float(np.float32(A_LAT / 4.0))      # sv2 = Sin(SV2*kx)
SU2 = float(np.float32(0.75 * A_CC))      # s3u2 = Sin(SU2*ky)
SQ_SCALE = float(np.float32(-8.0 * T_EV * T_EV))   # r = Sqrt(SQ_SCALE*gp2 + SQ_BIAS)
SQ_BIAS = float(np.float32(M_EV * M_EV + T_EV * T_EV))
H_A = float(np.float32(8.0 * TP_EV * EV_TO_J))     # h = H_A*gp2 + H_B
H_B = float(np.float32(2.0 * TP_EV * EV_TO_J))
J = float(np.float32(EV_TO_J))

_CACHE = {}
LAST_RESULTS = None


def _build():
    import concourse.bacc as bacc
    import concourse.tile as tile
    from concourse import mybir

    AF = mybir.ActivationFunctionType
    ALU = mybir.AluOpType
    f32 = mybir.dt.float32

    nc = bacc.Bacc(None, target_bir_lowering=False)

    def register_const(value):
        t = nc.alloc_sbuf_tensor(f"const-f32-{value}", [128, 1], f32)
        nc.gpsimd.memset(t.ap(), value)
        nc.const_aps.aps[(f32, value)] = t.ap()

    register_const(SQ_BIAS)
    nc.all_engine_barrier()

    k = nc.dram_tensor("k", [N_CORE, 2], f32, kind="ExternalInput")
    out = nc.dram_tensor("out", [N_CORE, 2], f32, kind="ExternalOutput")
    kv = k.rearrange("(t p f) two -> t p (f two)", p=P, f=F)
    ov = out.rearrange("(t p f) two -> t p (f two)", p=P, f=F)

    with tile.TileContext(nc) as tc:
        with (
            tc.tile_pool(name="kin", bufs=3) as kin_pool,
            tc.tile_pool(name="trig", bufs=2) as trig_pool,
            tc.tile_pool(name="gp", bufs=T) as gp_pool,
            tc.tile_pool(name="phb", bufs=3) as phb_pool,
            tc.tile_pool(name="outp", bufs=3) as out_pool,
        ):
            FB = F                          # phase-B sub-tile width
            gp2s = []
            # ---- phase A: trig table (Sin on ACT; squares split ACT/DVE) ----
            for t in range(T):
                kt = kin_pool.tile([P, 2 * F], f32, tag="kin")
                nc.sync.dma_start(kt[:], kv[t])
                kk = kt.rearrange("p (f two) -> p f two", two=2)
                sv2 = trig_pool.tile([P, F], f32, tag="sv2")
                s3u2 = trig_pool.tile([P, F], f32, tag="s3u2")
                nc.scalar.activation(sv2[:], kk[:, :, 0], AF.Sin, scale=SV2)
                nc.scalar.activation(s3u2[:], kk[:, :, 1], AF.Sin, scale=SU2)
                a = trig_pool.tile([P, F], f32, tag="a")
                nc.scalar.activation(a[:], sv2[:], AF.Square)   # a = sv2^2 (ACT)
                b = s3u2                                        # b = s3u2^2 (DVE)
                nc.vector.tensor_mul(b[:], s3u2[:], s3u2[:])
                w = b                                           # w = a+b (in-place)
                nc.vector.tensor_add(w[:], a[:], b[:])
                p = a                                           # p = 1-2a (in-place)
                nc.vector.tensor_scalar(p[:], a[:], -2.0, 1.0,
                                        ALU.mult, ALU.add)
                gp2 = gp_pool.tile([P, F], f32)
                nc.vector.scalar_tensor_tensor(gp2[:], w[:], 1.0, p[:],
                                               ALU.subtract, ALU.mult)
                gp2s.append(gp2)                                # (w-1)*p = -gp/2
            # ---- phase B: sqrt table; finer sub-tiles to stream out early ----
            for t in range(T):
                for s in range(F // FB):
                    gp2 = gp2s[t][:, s * FB:(s + 1) * FB]
                    h = phb_pool.tile([P, FB], f32, tag="h")
                    nc.scalar.activation(h[:], gp2, AF.Copy,
                                         bias=H_B, scale=H_A)
                    r = phb_pool.tile([P, FB], f32, tag="r")
                    nc.scalar.activation(r[:], gp2, AF.Sqrt,
                                         bias=SQ_BIAS, scale=SQ_SCALE)
                    ot = out_pool.tile([P, 2 * FB], f32, tag="ot")
                    oo = ot.rearrange("p (f two) -> p f two", two=2)
                    nc.vector.scalar_tensor_tensor(oo[:, :, 0], r[:], -J, h[:],
                                                   ALU.mult, ALU.add)
                    nc.vector.scalar_tensor_tensor(oo[:, :, 1], r[:], J, h[:],
                                                   ALU.mult, ALU.add)
                    nc.sync.dma_start(ov[t][:, 2 * s * FB:2 * (s + 1) * FB], ot[:])
    nc.compile()
    return nc


def kernel(k: np.ndarray) -> np.ndarray:
    global LAST_RESULTS
    from concourse.bass_utils import run_bass_kernel_spmd

    if "nc" not in _CACHE:
        _CACHE["nc"] = _build()
    nc = _CACHE["nc"]

    k = np.ascontiguousarray(np.asarray(k, dtype=np.float32))
    in_maps = [{"k": k[i * N_CORE:(i + 1) * N_CORE]} for i in range(N_CORES)]
    res = run_bass_kernel_spmd(nc, in_maps, list(range(N_CORES)),
                               trace=bool(os.environ.get("GRAPHENE_TRACE")))
    LAST_RESULTS = res
    return np.concatenate([res.results[i]["out"] for i in range(N_CORES)], axis=0)
